# revision 35
# baseline (speedup 1.0000x reference)
"""Trainium2 Bass kernel for nn_CCL__69277822485245 (spectral conv via DCT/FFT).

Math: the reference's rFFT along W cancels into a circular 5-tap convolution,
and the DCT-II sandwich M @ diag(D[:,s]) @ D collapses into 5 dense 128x128
matrices G_s (precomputed on host). Per batch element:

    u_s[i, m, w] = sum_h G_s[m, h] x[i, h, w]                  (stage 1)
    out[o, m, n] = sum_{s,t,i} W[o,i,s,t] u_s[i, m, (n-t)%W] + bias[o]

Sharding: data-parallel over batch B=8 across the 8 NeuronCores (1 each).

v2 layout — w-parity packing (no duplication, no w-halo in stage 1):
  stage 1: lhsT = x2[h=128, (w-pair jp -> 128 cols: w=2jp i0..63, w=2jp+1
      i0..63)] (stationary, one load per jp), rhs = gt[h, (mh, s, m)] N=320.
      psum[(wp,i), (s,m)] -> one straight (non-transposing) copy per (jp,mh)
      into u[(wp,i), s, HALO+jp, m]; jp 62,63 also copied to the front halo
      slots (circular W).
  stage 2: output n split by parity p; kernel taps t pair across partition
      halves by w-parity of n-t. Per (s,p): two K=128 pairs + one K=64 solo,
      each a jp-offset slice of u. 15 accumulating matmuls per psum chunk,
      chunk = [o=128, (jp=64, m=8)] so finished output is contiguous per
      m-row -> efficient streaming DMA out per 8-m block.

DTYPE "bf16": 1 cyc/row matmuls, rel err ~ 3e-3 (gate 2e-2).
"""

import numpy as np

H = 128
W = 128
CI = 64
CO = 128
KH = 5
KW = 5
B = 8

MH = 64          # m-half processed per outer iteration
JP = W // 2      # 64 w-pairs
HALO = 2         # front jp-halo (circular W wrap for t-shifts)
JX = HALO + JP   # 66

DTYPE = "bf16"
# x col chunks as (offset, size): small first chunks -> stage 1 starts
# sooner.  Chunk 0 = cols for jp 62,63 (the circular-wrap halo sources);
# jp 62,63 run FIRST so their u writes + halo copies happen early, not at
# the stage-1 tail where stage 2 waits on them.  Chunks alternate between
# the two HWDGE queues (SP / Activation): phase-1 consumes ~204 ns/jp,
# faster than one queue can stream.
JPORDER = (62, 63) + tuple(range(JP - 2))
# x columns are PERMUTED on host into JPORDER order (RANK[jp] = position),
# so chunks are consumed strictly sequentially: early chunks small (launch
# latency), later chunks big (DMA packet efficiency), alternating HWDGE
# queues since phase-1 consumes ~204 ns/jp, faster than one queue streams.
RANK = {jp: r for r, jp in enumerate(JPORDER)}
XCHUNKS = ((0, 128), (128, 128), (256, 512), (768, 512), (1280, 1024),
           (2304, 1024), (3328, 1024), (4352, 1536), (5888, 1280),
           (7168, 1024))
XQUEUE = ("sync", "sync", "sync", "scalar", "sync", "gpsimd",
          "sync", "scalar", "sync", "gpsimd")

_PROG = None
_CONSTS = None
_RUN_OPTS = {}     # test harness may set e.g. {"trace": True, "trace_cores": [0]}
_LAST_RESULT = None

# stage-2 tap-pair groups per parity: per s, two K=128 matmuls with taps
# paired across the w-parity (d) partition halves:
#   p=0: (t2|t1) off -1, (t4|t3) off -2; leftover solo tap t=0, d=0, off  0
#   p=1: (t1|t0) off  0, (t3|t2) off -1; leftover solo tap t=4, d=1, off -2
_PAIRS = {0: [(-1, 0, 128), (-2, 0, 128)], 1: [(0, 0, 128), (-1, 0, 128)]}
# the 5 leftover solo taps (K=64 each) are s-PAIRED into K=128 matmuls using
# SP tiles (partition-shuffled copies of u: rows 0:64 = s even, 64:128 = s
# odd, for s pairs q=(0,1),(2,3)); s=4 remains a true K=64 solo from u.
#   p -> (solo tap t, u kbase (d-half), jp offset)
_SOLO = {0: (0, 0, 0), 1: (4, 64, -2)}
NG = 13          # chain groups per parity: 10 pairs + 2 sp-pairs + s4 solo
MH_SPLIT = True  # pipeline stage-1 mh1 into stage2(mh0) chains


def _np_dt():
    if DTYPE == "bf16":
        import ml_dtypes
        return ml_dtypes.bfloat16
    return np.float32


def _build_consts():
    n = np.arange(H, dtype=np.float64)
    ang = np.pi * (2.0 * n[None, :] + 1.0) * n[:, None] / (2.0 * H)  # [k, h]
    D = 2.0 * np.cos(ang)
    wgt = np.where(n == 0, 0.5, 1.0)
    M = (np.cos(ang).T * wgt[None, :]) / (2.0 * H)                    # [m, k]
    G = np.stack([M @ (D[:, s:s + 1] * D) for s in range(KH)])        # [s, m, h]
    # gt layout [h, (mh, s, m)]: col = mh*320 + s*64 + ml
    GT = (G.transpose(2, 0, 1)                # [h, s, m]
            .reshape(H, KH, 2, MH)            # [h, s, mh, ml]
            .transpose(0, 2, 1, 3)            # [h, mh, s, ml]
            .reshape(H, KH * H))
    return np.ascontiguousarray(GT).astype(_np_dt())


def _build_wstack(weight):
    # wst[(d or z, i), (p, bi, o)] with 13 col-blocks bi per parity:
    #   bi 0..9  = tap-pair groups (s*2+g), rows = t-pair across d halves
    #   bi 10,11 = s-paired solos q=0,1: rows 0:64 = W[:,:,2q,t], 64: = 2q+1
    #   bi 12    = s4 solo (K=64 at its d-half kbase)
    wst = np.zeros((128, 2 * NG * CO), np.float32)
    col = 0
    for p in range(2):
        pairs = [(2, 1), (4, 3)] if p == 0 else [(1, 0), (3, 2)]
        tsolo, kbsolo, _ = _SOLO[p]
        for s in range(KH):
            Wl = weight[:, :, s, :]          # [o, i, t]
            for tl, tu in pairs:
                wst[0:64, col:col + CO] = Wl[:, :, tl].T
                wst[64:128, col:col + CO] = Wl[:, :, tu].T
                col += CO
        for q in range(2):
            wst[0:64, col:col + CO] = weight[:, :, 2 * q, tsolo].T
            wst[64:128, col:col + CO] = weight[:, :, 2 * q + 1, tsolo].T
            col += CO
        wst[kbsolo:kbsolo + 64, col:col + CO] = weight[:, :, 4, tsolo].T
        col += CO
    return np.ascontiguousarray(wst).astype(_np_dt())


def _build_program():
    import concourse.mybir as mybir
    import concourse.tile as tile
    from concourse import bacc

    f32 = mybir.dt.float32
    mmdt = {"bf16": mybir.dt.bfloat16,
            "f32r": mybir.dt.float32r,
            "f32": mybir.dt.float32}[DTYPE]

    nc = bacc.Bacc("TRN2", target_bir_lowering=False, debug=False,
                   enable_asserts=False, num_devices=B)
    x_ds = [nc.dram_tensor(f"x{c}", [H, sz], mmdt,
                           kind="ExternalInput").ap()
            for c, (_, sz) in enumerate(XCHUNKS)]
    # g0 split in two so the very first matmul gates on only 32 KB
    g_ds = [nc.dram_tensor("g0a", [H, 128], mmdt, kind="ExternalInput").ap(),
            nc.dram_tensor("g0b", [H, KH * MH - 128], mmdt,
                           kind="ExternalInput").ap(),
            nc.dram_tensor("g1", [H, KH * MH], mmdt,
                           kind="ExternalInput").ap()]
    w_d = nc.dram_tensor("wt", [128, 2 * NG * CO], mmdt,
                         kind="ExternalInput").ap()
    b_d = nc.dram_tensor("bias", [CO, 1], f32, kind="ExternalInput").ap()
    o_d = nc.dram_tensor("out", [CO, H, W], mmdt, kind="ExternalOutput").ap()

    with tile.TileContext(nc) as tc:
        with (
            tc.tile_pool(name="const", bufs=1) as cpool,
            tc.tile_pool(name="u", bufs=1) as upool,
            tc.tile_pool(name="oacc", bufs=1) as opool,
            tc.tile_pool(name="ps", bufs=1, space="PSUM") as psp,
        ):
            import concourse.mybir as _mb

            xt = cpool.tile([H, W * CI], mmdt)
            # DMA split across both HWDGE queues: x chunks on the SP (sync)
            # queue, constants on the Activation (scalar) queue, so the first
            # matmul's gates (g-half0 on scalar + x chunk0 on sync) transfer
            # in parallel.
            gt = cpool.tile([H, KH * H], mmdt)
            nc.scalar.dma_start(gt[:, 0:128], g_ds[0])
            nc.scalar.dma_start(gt[:, 128:KH * MH], g_ds[1])
            for c in range(len(XCHUNKS)):
                off, sz = XCHUNKS[c]
                eng = {"sync": nc.sync, "scalar": nc.scalar,
                       "gpsimd": nc.gpsimd}[XQUEUE[c]]
                eng.dma_start(xt[:, off:off + sz], x_ds[c])
            nc.scalar.dma_start(gt[:, KH * MH:KH * H], g_ds[2])
            wt = cpool.tile([128, 2 * NG * CO], mmdt)
            nc.scalar.dma_start(wt[:], w_d)
            bt = cpool.tile([CO, 1], f32)
            nc.scalar.dma_start(bt[:], b_d)

            def mm(out, lhsT, rhs, start, stop, reload):
                inst = nc.tensor.matmul(out, lhsT, rhs, start=start, stop=stop)
                if not reload:      # stationary weights already in the array
                    inst.ldweights = False

            def s1_batch(mh, ja, jb, tag, eng):
                # one psum tile, two 320-col matmuls (jp pair ja, jb=ja+1 of
                # the given mh half), one 640-col evac.  jp 62,63 also get
                # the circular front-halo copy on the idle gpsimd engine.
                pt = psp.tile([128, 1024], f32, tag=f"ps{tag}",
                              name=f"ps{tag}")
                for bi, jp in ((0, ja), (1, jb)):
                    lhsT = xt[:, RANK[jp] * 128:(RANK[jp] + 1) * 128]
                    if mh == 0 and ja == JPORDER[0]:
                        # first batch: two mms split on the g0a/g0b DMA
                        # boundary so the kernel starts on 32 KB of g, not 80
                        mm(pt[:, bi * 512:bi * 512 + 128], lhsT,
                           gt[:, 0:128], start=True, stop=True, reload=True)
                        mm(pt[:, bi * 512 + 128:bi * 512 + KH * MH], lhsT,
                           gt[:, 128:KH * MH],
                           start=True, stop=True, reload=False)
                    else:
                        mm(pt[:, bi * 512:bi * 512 + KH * MH], lhsT,
                           gt[:, mh * KH * MH:(mh + 1) * KH * MH],
                           start=True, stop=True, reload=True)
                pv = (pt[:].rearrange("p (b q) -> p b q", b=2)
                      [:, :, 0:KH * MH]
                      .rearrange("p b (s m) -> p b s m", s=KH))
                dst = (u5[:, mh, :, HALO + ja:HALO + ja + 2, :]
                       .transpose([0, 2, 1, 3]))
                if eng == 0:
                    nc.vector.tensor_copy(dst, pv)
                else:
                    nc.scalar.activation(dst, pv,
                                         _mb.ActivationFunctionType.Identity)
                if ja == JP - 2:
                    nc.gpsimd.tensor_copy(u5[:, mh, :, 0:2, :],
                                          u5[:, mh, :, HALO + ja:HALO + JP, :])

            def s1_jp(jp, tag, eng):
                # both-mh stage-1 for one jp: single lhsT load amortized
                # over two 320-col matmuls, one 640-col evac
                pt = psp.tile([128, 1024], f32, tag=f"ps{tag}",
                              name=f"ps{tag}")
                for mh in range(2):
                    mm(pt[:, mh * 512:mh * 512 + KH * MH],
                       xt[:, RANK[jp] * 128:(RANK[jp] + 1) * 128],
                       gt[:, mh * KH * MH:(mh + 1) * KH * MH],
                       start=True, stop=True, reload=(mh == 0))
                pv = (pt[:].rearrange("p (h q) -> p h q", h=2)
                      [:, :, 0:KH * MH]
                      .rearrange("p h (s m) -> p h s m", s=KH))
                if eng == 0:
                    nc.vector.tensor_copy(u5[:, :, :, HALO + jp, :], pv)
                else:
                    nc.scalar.activation(u5[:, :, :, HALO + jp, :], pv,
                                         _mb.ActivationFunctionType.Identity)
                if jp >= JP - HALO:
                    nc.gpsimd.tensor_copy(u5[:, :, :, jp - (JP - HALO), :],
                                          u5[:, :, :, HALO + jp, :])

            S1BATCHES = [(JPORDER[i], JPORDER[i + 1])
                         for i in range(0, JP, 2)]

            def sp_fills(mh, sp0v, sp1v):
                # s-paired (z = s parity -> partition half) SBUF->SBUF copies
                # of u's solo-tap data for this mh half, on the SP DMA queue
                u4s = u[:].rearrange("p (h s j m) -> p h s j m",
                                     h=2, s=KH, j=JX)
                for w0, w1 in ((0, JP // 2), (JP // 2, JP)):
                    for z in range(2):
                        src = (u4s[0:64, mh, 0:4, HALO + w0:HALO + w1, :]
                               .rearrange("p (q z) j m -> p q z j m", z=2)
                               [:, :, z, :, :])
                        nc.sync.dma_start(
                            sp0v[z * 64:(z + 1) * 64, mh, :, w0:w1, :], src)
                for w0, w1 in ((0, JX // 2), (JX // 2, JX)):
                    for z in range(2):
                        src = (u4s[64:128, mh, 0:4, w0:w1, :]
                               .rearrange("p (q z) j m -> p q z j m", z=2)
                               [:, :, z, :, :])
                        nc.sync.dma_start(
                            sp1v[z * 64:(z + 1) * 64, mh, :, w0:w1, :], src)


            def stage2(u4, mh, sp0v, sp1v, extras=None):
                # extras: queue of thunks (interleaved stage-1 mh1 batches),
                # emitted mid-chain at gi points where the OTHER psum tag
                # pair (previous chain's, already evacuated) is free
                extras = list(extras or [])
                for p in range(2):
                    tsolo, kbs, soff = _SOLO[p]
                    spv = sp0v if p == 0 else sp1v
                    spj = 0 if p == 0 else HALO + soff   # j base in SP tile
                    # half-sweep of 4 m-chunks = 2 psum tiles (2 chunks per
                    # tile at bank-aligned halves) -> runs of 4 same-lhsT
                    # matmuls; tags (0,1)/(2,3) alternate across half-sweeps
                    for hs in range(2):
                        tags = (2 * hs, 2 * hs + 1)
                        otags = (2 * (1 - hs), 2 * (1 - hs) + 1)
                        p2s = [psp.tile([128, 1024], f32, tag=f"ps{t}",
                                        name=f"ps{t}") for t in tags]
                        oacc = opool.tile([CO, 4 * 8 * W], mmdt,
                                          tag=f"oacc{(mh * 2 + hs) % 3}")
                        # [o, m-local, jn-pair, parity]
                        oa4 = oacc[:].rearrange("p (m j q) -> p m j q",
                                                m=32, q=2)
                        # chain: s4 solo (K=64 from u), 10 tap-pairs, then
                        # the two s-paired solos LAST (max SP-fill slack)
                        chain = [("s4",)] + \
                            [("pair", s, g) for s in range(KH)
                             for g in range(2)] + \
                            [("sp", 0), ("sp", 1)]
                        for gi, grp in enumerate(chain):
                            if grp[0] == "pair":
                                s, g = grp[1], grp[2]
                                off, kb, kk = _PAIRS[p][g]
                                bi = s * 2 + g
                            elif grp[0] == "sp":
                                bi = 10 + grp[1]
                                kb, kk = 0, 128
                            else:
                                off, kb, kk = soff, kbs, 64
                                bi = 12
                            gb = p * NG + bi
                            lhsT = wt[kb:kb + kk, gb * CO:(gb + 1) * CO]
                            for k in range(4):
                                mc = hs * 4 + k
                                if grp[0] == "sp":
                                    rhs = spv[:, mh, grp[1],
                                              spj:spj + JP,
                                              mc * 8:(mc + 1) * 8]
                                elif grp[0] == "s4":
                                    rhs = u4[kb:kb + kk, 4,
                                             HALO + soff:HALO + soff + JP,
                                             mc * 8:(mc + 1) * 8]
                                else:
                                    rhs = u4[kb:kb + kk, s,
                                             HALO + off:HALO + off + JP,
                                             mc * 8:(mc + 1) * 8]
                                mm(p2s[k // 2][:, (k % 2) * 512:
                                               (k % 2) * 512 + 512],
                                   lhsT, rhs, start=(gi == 0),
                                   stop=(gi == NG - 1), reload=(k == 0))
                            # interleave two stage-1 mh1 batches per
                            # point on the free (previous chain's, already-
                            # evacuated) psum tag pair; the final chain uses
                            # early points so its last batch's evac lands
                            # well before stage2(mh1) starts
                            pts = (5, 6, 7, 8) if (p == 1 and hs == 1) \
                                else (5, 7, 9, 11)
                            if extras and gi in pts:
                                extras.pop(0)(otags[0])
                                if extras:
                                    extras.pop(0)(otags[1])
                        evacs, trigs = [], []
                        fin = (mh == 1 and p == 1 and hs == 1)
                        for k in range(4):
                            mc = hs * 4 + k
                            psl = (p2s[k // 2][:, (k % 2) * 512:
                                               (k % 2) * 512 + 512]
                                   .rearrange("p (j m) -> p j m", j=JP))
                            # very last psum chunk: halve the evac across
                            # both engines + both DMA queues to shorten the
                            # kernel tail
                            halves = ((0, 4), (4, 8)) if (fin and k == 3) \
                                else ((0, 8),)
                            for hi, (m0, m1) in enumerate(halves):
                                lm = k * 8 + m0
                                tgt = oa4[:, lm:lm + (m1 - m0), :, p]
                                src = psl[:, :, m0:m1].transpose([0, 2, 1])
                                if (k + hi) % 2 == 0:
                                    evacs.append(lambda t=tgt, s=src:
                                                 nc.scalar.activation(
                                                     t, s,
                                                     _mb.ActivationFunctionType
                                                     .Identity, bias=bt[:]))
                                else:
                                    evacs.append(lambda t=tgt, s=src:
                                                 nc.vector.tensor_scalar_add(
                                                     t, s, bt[:]))
                                if p == 1:   # both parities done -> stream;
                                    # triggers split across HWDGE queues,
                                    # emitted after ALL evacs so the scalar
                                    # engine never delays an evac for one
                                    deng = nc.sync if (k + hi) % 2 == 0 \
                                        else nc.scalar
                                    trigs.append(
                                        lambda d=deng, mc=mc, lm=lm,
                                        m0=m0, m1=m1:
                                        d.dma_start(
                                            o_d[:, mh * MH + mc * 8 + m0:
                                                mh * MH + mc * 8 + m1, :],
                                            oacc[:, lm * W:
                                                 (lm + m1 - m0) * W]))
                        for e in evacs:
                            e()
                        for t in trigs:
                            t()
                assert not extras

            u = upool.tile([128, 2 * KH * JX * MH], mmdt)
            u5 = u[:].rearrange("p (h s j m) -> p h s j m", h=2, s=KH, j=JX)

            # SP tiles: s-paired (z = s parity across partition halves)
            # copies of u's solo-tap data, DMA-filled per mh half once that
            # half's stage-1 writes land.  SP0: d=0 (p=0 solo, t=0, no
            # halo); SP1: d=1 (p=1 solo, t=4, keeps front halo).
            sp0 = upool.tile([128, 2 * 2 * JP * MH], mmdt)
            sp1 = upool.tile([128, 2 * 2 * JX * MH], mmdt)
            sp0v = sp0[:].rearrange("p (h q j m) -> p h q j m", h=2, q=2, j=JP)
            sp1v = sp1[:].rearrange("p (h q j m) -> p h q j m", h=2, q=2, j=JX)

            if MH_SPLIT:
                # phase 1: stage-1 for mh0 only (evac-throughput-bound
                # window is half as long); mh1's stage-1 batches run INSIDE
                # stage2(mh0)'s chains where the evac engines have slack
                for bn, (ja, jb) in enumerate(S1BATCHES):
                    s1_batch(0, ja, jb, bn % 4, bn % 2)
                sp_fills(0, sp0v, sp1v)
                mh1_thunks = [
                    (lambda t, ja=ja, jb=jb, e=bn % 2:
                     s1_batch(1, ja, jb, t, e))
                    for bn, (ja, jb) in enumerate(S1BATCHES)]
                stage2(u5[:, 0], 0, sp0v, sp1v, extras=mh1_thunks)
                sp_fills(1, sp0v, sp1v)
                stage2(u5[:, 1], 1, sp0v, sp1v)
            else:
                for n, jp in enumerate(JPORDER):
                    s1_jp(jp, n % 4, n % 2)
                sp_fills(0, sp0v, sp1v)
                sp_fills(1, sp0v, sp1v)
                stage2(u5[:, 0], 0, sp0v, sp1v)
                stage2(u5[:, 1], 1, sp0v, sp1v)
    _strip_redundant_ldweights(nc)
    nc.compile()
    return nc


def _strip_redundant_ldweights(nc):
    """Drop InstLdweights whose weights AP equals the previous load on the
    tensor queue: the PE array still holds those weights (nothing between
    two same-AP loads rewrites that SBUF region in this kernel), so the
    reload is pure overhead (~100ns each, serialized with the matmuls)."""

    def sig(ap):
        return str(ap)

    removed = kept = 0
    for fn in nc.m.functions:
        for bb in fn.blocks:
            insts = bb.instructions
            last = None
            for inst in list(insts):
                nm = type(inst).__name__
                if nm == "InstLdweights":
                    s = sig(inst.ins[0])
                    si = inst.sync_info
                    clean = si is None or (not si.on_wait and not si.on_update)
                    if s == last and clean:
                        insts.remove(inst)
                        removed += 1
                    else:
                        last = s
                        kept += 1
                elif nm == "InstMatmult":
                    pass          # matmuls leave the stationary weights alone
    return removed, kept


def _get_prog():
    global _PROG
    if _PROG is None:
        _PROG = _build_program()
    return _PROG


def kernel(x, weight, bias):
    from concourse.bass_utils import run_bass_kernel_spmd

    global _CONSTS
    if _CONSTS is None:
        _CONSTS = _build_consts()
    GT = _CONSTS

    x = np.ascontiguousarray(np.asarray(x, dtype=np.float32))
    weight = np.ascontiguousarray(np.asarray(weight, dtype=np.float32))
    bias = np.ascontiguousarray(np.asarray(bias, dtype=np.float32))

    wst = _build_wstack(weight)
    b2 = np.ascontiguousarray(bias.reshape(CO, 1))

    in_maps = []
    for b in range(B):
        # x2[h, (w, i)] with jp col-blocks PERMUTED into JPORDER order so
        # DMA chunks stream in exact device consumption order
        x2 = (x[b].transpose(1, 2, 0).reshape(H, JP, 2 * CI)
              [:, list(JPORDER), :].reshape(H, W * CI)).astype(_np_dt())
        m = {}
        for c, (off, sz) in enumerate(XCHUNKS):
            m[f"x{c}"] = np.ascontiguousarray(x2[:, off:off + sz])
        m.update({"g0a": np.ascontiguousarray(GT[:, 0:128]),
                  "g0b": np.ascontiguousarray(GT[:, 128:KH * MH]),
                  "g1": np.ascontiguousarray(GT[:, KH * MH:]),
                  "wt": wst, "bias": b2})
        in_maps.append(m)

    res = run_bass_kernel_spmd(_get_prog(), in_maps, core_ids=list(range(B)),
                               **_RUN_OPTS)
    global _LAST_RESULT
    _LAST_RESULT = res
    out = np.stack([res.results[b]["out"] for b in range(B)], axis=0)
    return np.ascontiguousarray(out.astype(np.float32))



# revision 36
# speedup vs baseline: 1.0127x; 1.0127x over previous
"""Trainium2 Bass kernel for nn_CCL__69277822485245 (spectral conv via DCT/FFT).

Math: the reference's rFFT along W cancels into a circular 5-tap convolution,
and the DCT-II sandwich M @ diag(D[:,s]) @ D collapses into 5 dense 128x128
matrices G_s (precomputed on host). Per batch element:

    u_s[i, m, w] = sum_h G_s[m, h] x[i, h, w]                  (stage 1)
    out[o, m, n] = sum_{s,t,i} W[o,i,s,t] u_s[i, m, (n-t)%W] + bias[o]

Sharding: data-parallel over batch B=8 across the 8 NeuronCores (1 each).

v2 layout — w-parity packing (no duplication, no w-halo in stage 1):
  stage 1: lhsT = x2[h=128, (w-pair jp -> 128 cols: w=2jp i0..63, w=2jp+1
      i0..63)] (stationary, one load per jp), rhs = gt[h, (mh, s, m)] N=320.
      psum[(wp,i), (s,m)] -> one straight (non-transposing) copy per (jp,mh)
      into u[(wp,i), s, HALO+jp, m]; jp 62,63 also copied to the front halo
      slots (circular W).
  stage 2: output n split by parity p; kernel taps t pair across partition
      halves by w-parity of n-t. Per (s,p): two K=128 pairs + one K=64 solo,
      each a jp-offset slice of u. 15 accumulating matmuls per psum chunk,
      chunk = [o=128, (jp=64, m=8)] so finished output is contiguous per
      m-row -> efficient streaming DMA out per 8-m block.

DTYPE "bf16": 1 cyc/row matmuls, rel err ~ 3e-3 (gate 2e-2).
"""

import numpy as np

H = 128
W = 128
CI = 64
CO = 128
KH = 5
KW = 5
B = 8

MH = 64          # m-half processed per outer iteration
JP = W // 2      # 64 w-pairs
HALO = 2         # front jp-halo (circular W wrap for t-shifts)
JX = HALO + JP   # 66

DTYPE = "bf16"
# x col chunks as (offset, size): small first chunks -> stage 1 starts
# sooner.  Chunk 0 = cols for jp 62,63 (the circular-wrap halo sources);
# jp 62,63 run FIRST so their u writes + halo copies happen early, not at
# the stage-1 tail where stage 2 waits on them.  Chunks alternate between
# the two HWDGE queues (SP / Activation): phase-1 consumes ~204 ns/jp,
# faster than one queue can stream.
JPORDER = (62, 63) + tuple(range(JP - 2))
# x columns are PERMUTED on host into JPORDER order (RANK[jp] = position),
# so chunks are consumed strictly sequentially: early chunks small (launch
# latency), later chunks big (DMA packet efficiency), alternating HWDGE
# queues since phase-1 consumes ~204 ns/jp, faster than one queue streams.
RANK = {jp: r for r, jp in enumerate(JPORDER)}
XCHUNKS = ((0, 128), (128, 128), (256, 512), (768, 512), (1280, 1024),
           (2304, 1024), (3328, 1024), (4352, 1536), (5888, 1280),
           (7168, 1024))
XQUEUE = ("sync", "sync", "sync", "scalar", "sync", "scalar",
          "sync", "scalar", "sync", "scalar")

_PROG = None
_CONSTS = None
_RUN_OPTS = {}     # test harness may set e.g. {"trace": True, "trace_cores": [0]}
_LAST_RESULT = None

# stage-2 tap-pair groups per parity: per s, two K=128 matmuls with taps
# paired across the w-parity (d) partition halves:
#   p=0: (t2|t1) off -1, (t4|t3) off -2; leftover solo tap t=0, d=0, off  0
#   p=1: (t1|t0) off  0, (t3|t2) off -1; leftover solo tap t=4, d=1, off -2
_PAIRS = {0: [(-1, 0, 128), (-2, 0, 128)], 1: [(0, 0, 128), (-1, 0, 128)]}
# the 5 leftover solo taps (K=64 each) are s-PAIRED into K=128 matmuls using
# SP tiles (partition-shuffled copies of u: rows 0:64 = s even, 64:128 = s
# odd, for s pairs q=(0,1),(2,3)); s=4 remains a true K=64 solo from u.
#   p -> (solo tap t, u kbase (d-half), jp offset)
_SOLO = {0: (0, 0, 0), 1: (4, 64, -2)}
NG = 13          # chain groups per parity: 10 pairs + 2 sp-pairs + s4 solo
MH_SPLIT = True  # pipeline stage-1 mh1 into stage2(mh0) chains


def _np_dt():
    if DTYPE == "bf16":
        import ml_dtypes
        return ml_dtypes.bfloat16
    return np.float32


def _build_consts():
    n = np.arange(H, dtype=np.float64)
    ang = np.pi * (2.0 * n[None, :] + 1.0) * n[:, None] / (2.0 * H)  # [k, h]
    D = 2.0 * np.cos(ang)
    wgt = np.where(n == 0, 0.5, 1.0)
    M = (np.cos(ang).T * wgt[None, :]) / (2.0 * H)                    # [m, k]
    G = np.stack([M @ (D[:, s:s + 1] * D) for s in range(KH)])        # [s, m, h]
    # gt layout [h, (mh, s, m)]: col = mh*320 + s*64 + ml
    GT = (G.transpose(2, 0, 1)                # [h, s, m]
            .reshape(H, KH, 2, MH)            # [h, s, mh, ml]
            .transpose(0, 2, 1, 3)            # [h, mh, s, ml]
            .reshape(H, KH * H))
    return np.ascontiguousarray(GT).astype(_np_dt())


def _build_wstack(weight):
    # wst[(d or z, i), (p, bi, o)] with 13 col-blocks bi per parity:
    #   bi 0..9  = tap-pair groups (s*2+g), rows = t-pair across d halves
    #   bi 10,11 = s-paired solos q=0,1: rows 0:64 = W[:,:,2q,t], 64: = 2q+1
    #   bi 12    = s4 solo (K=64 at its d-half kbase)
    wst = np.zeros((128, 2 * NG * CO), np.float32)
    col = 0
    for p in range(2):
        pairs = [(2, 1), (4, 3)] if p == 0 else [(1, 0), (3, 2)]
        tsolo, kbsolo, _ = _SOLO[p]
        for s in range(KH):
            Wl = weight[:, :, s, :]          # [o, i, t]
            for tl, tu in pairs:
                wst[0:64, col:col + CO] = Wl[:, :, tl].T
                wst[64:128, col:col + CO] = Wl[:, :, tu].T
                col += CO
        for q in range(2):
            wst[0:64, col:col + CO] = weight[:, :, 2 * q, tsolo].T
            wst[64:128, col:col + CO] = weight[:, :, 2 * q + 1, tsolo].T
            col += CO
        wst[kbsolo:kbsolo + 64, col:col + CO] = weight[:, :, 4, tsolo].T
        col += CO
    return np.ascontiguousarray(wst).astype(_np_dt())


def _build_program():
    import concourse.mybir as mybir
    import concourse.tile as tile
    from concourse import bacc

    f32 = mybir.dt.float32
    mmdt = {"bf16": mybir.dt.bfloat16,
            "f32r": mybir.dt.float32r,
            "f32": mybir.dt.float32}[DTYPE]

    nc = bacc.Bacc("TRN2", target_bir_lowering=False, debug=False,
                   enable_asserts=False, num_devices=B)
    x_ds = [nc.dram_tensor(f"x{c}", [H, sz], mmdt,
                           kind="ExternalInput").ap()
            for c, (_, sz) in enumerate(XCHUNKS)]
    # g0 split in two so the very first matmul gates on only 32 KB
    g_ds = [nc.dram_tensor("g0a", [H, 128], mmdt, kind="ExternalInput").ap(),
            nc.dram_tensor("g0b", [H, KH * MH - 128], mmdt,
                           kind="ExternalInput").ap(),
            nc.dram_tensor("g1", [H, KH * MH], mmdt,
                           kind="ExternalInput").ap()]
    w_d = nc.dram_tensor("wt", [128, 2 * NG * CO], mmdt,
                         kind="ExternalInput").ap()
    b_d = nc.dram_tensor("bias", [CO, 1], f32, kind="ExternalInput").ap()
    o_d = nc.dram_tensor("out", [CO, H, W], mmdt, kind="ExternalOutput").ap()

    with tile.TileContext(nc) as tc:
        with (
            tc.tile_pool(name="const", bufs=1) as cpool,
            tc.tile_pool(name="u", bufs=1) as upool,
            tc.tile_pool(name="oacc", bufs=1) as opool,
            tc.tile_pool(name="ps", bufs=1, space="PSUM") as psp,
        ):
            import concourse.mybir as _mb

            xt = cpool.tile([H, W * CI], mmdt)
            # DMA split across both HWDGE queues: x chunks on the SP (sync)
            # queue, constants on the Activation (scalar) queue, so the first
            # matmul's gates (g-half0 on scalar + x chunk0 on sync) transfer
            # in parallel.
            gt = cpool.tile([H, KH * H], mmdt)
            nc.scalar.dma_start(gt[:, 0:128], g_ds[0])
            nc.scalar.dma_start(gt[:, 128:KH * MH], g_ds[1])
            for c in range(len(XCHUNKS)):
                off, sz = XCHUNKS[c]
                eng = {"sync": nc.sync, "scalar": nc.scalar,
                       "gpsimd": nc.gpsimd}[XQUEUE[c]]
                eng.dma_start(xt[:, off:off + sz], x_ds[c])
            nc.scalar.dma_start(gt[:, KH * MH:KH * H], g_ds[2])
            wt = cpool.tile([128, 2 * NG * CO], mmdt)
            nc.scalar.dma_start(wt[:], w_d)
            bt = cpool.tile([CO, 1], f32)
            nc.scalar.dma_start(bt[:], b_d)

            def mm(out, lhsT, rhs, start, stop, reload):
                inst = nc.tensor.matmul(out, lhsT, rhs, start=start, stop=stop)
                if not reload:      # stationary weights already in the array
                    inst.ldweights = False

            def s1_batch(mh, ja, jb, tag, eng):
                # one psum tile, two 320-col matmuls (jp pair ja, jb=ja+1 of
                # the given mh half), one 640-col evac.  jp 62,63 also get
                # the circular front-halo copy on the idle gpsimd engine.
                pt = psp.tile([128, 1024], f32, tag=f"ps{tag}",
                              name=f"ps{tag}")
                for bi, jp in ((0, ja), (1, jb)):
                    lhsT = xt[:, RANK[jp] * 128:(RANK[jp] + 1) * 128]
                    if mh == 0 and ja == JPORDER[0]:
                        # first batch: two mms split on the g0a/g0b DMA
                        # boundary so the kernel starts on 32 KB of g, not 80
                        mm(pt[:, bi * 512:bi * 512 + 128], lhsT,
                           gt[:, 0:128], start=True, stop=True, reload=True)
                        mm(pt[:, bi * 512 + 128:bi * 512 + KH * MH], lhsT,
                           gt[:, 128:KH * MH],
                           start=True, stop=True, reload=False)
                    else:
                        mm(pt[:, bi * 512:bi * 512 + KH * MH], lhsT,
                           gt[:, mh * KH * MH:(mh + 1) * KH * MH],
                           start=True, stop=True, reload=True)
                pv = (pt[:].rearrange("p (b q) -> p b q", b=2)
                      [:, :, 0:KH * MH]
                      .rearrange("p b (s m) -> p b s m", s=KH))
                dst = (u5[:, mh, :, HALO + ja:HALO + ja + 2, :]
                       .transpose([0, 2, 1, 3]))
                if eng == 0:
                    nc.vector.tensor_copy(dst, pv)
                else:
                    nc.scalar.activation(dst, pv,
                                         _mb.ActivationFunctionType.Identity)
                if ja == JP - 2:
                    nc.gpsimd.tensor_copy(u5[:, mh, :, 0:2, :],
                                          u5[:, mh, :, HALO + ja:HALO + JP, :])

            def s1_jp(jp, tag, eng):
                # both-mh stage-1 for one jp: single lhsT load amortized
                # over two 320-col matmuls, one 640-col evac
                pt = psp.tile([128, 1024], f32, tag=f"ps{tag}",
                              name=f"ps{tag}")
                for mh in range(2):
                    mm(pt[:, mh * 512:mh * 512 + KH * MH],
                       xt[:, RANK[jp] * 128:(RANK[jp] + 1) * 128],
                       gt[:, mh * KH * MH:(mh + 1) * KH * MH],
                       start=True, stop=True, reload=(mh == 0))
                pv = (pt[:].rearrange("p (h q) -> p h q", h=2)
                      [:, :, 0:KH * MH]
                      .rearrange("p h (s m) -> p h s m", s=KH))
                if eng == 0:
                    nc.vector.tensor_copy(u5[:, :, :, HALO + jp, :], pv)
                else:
                    nc.scalar.activation(u5[:, :, :, HALO + jp, :], pv,
                                         _mb.ActivationFunctionType.Identity)
                if jp >= JP - HALO:
                    nc.gpsimd.tensor_copy(u5[:, :, :, jp - (JP - HALO), :],
                                          u5[:, :, :, HALO + jp, :])

            S1BATCHES = [(JPORDER[i], JPORDER[i + 1])
                         for i in range(0, JP, 2)]

            def sp_fills(mh, sp0v, sp1v):
                # s-paired (z = s parity -> partition half) SBUF->SBUF copies
                # of u's solo-tap data for this mh half, on the SP DMA queue
                u4s = u[:].rearrange("p (h s j m) -> p h s j m",
                                     h=2, s=KH, j=JX)
                for w0, w1 in ((0, JP // 2), (JP // 2, JP)):
                    for z in range(2):
                        src = (u4s[0:64, mh, 0:4, HALO + w0:HALO + w1, :]
                               .rearrange("p (q z) j m -> p q z j m", z=2)
                               [:, :, z, :, :])
                        nc.sync.dma_start(
                            sp0v[z * 64:(z + 1) * 64, mh, :, w0:w1, :], src)
                for w0, w1 in ((0, JX // 2), (JX // 2, JX)):
                    for z in range(2):
                        src = (u4s[64:128, mh, 0:4, w0:w1, :]
                               .rearrange("p (q z) j m -> p q z j m", z=2)
                               [:, :, z, :, :])
                        nc.sync.dma_start(
                            sp1v[z * 64:(z + 1) * 64, mh, :, w0:w1, :], src)


            def stage2(u4, mh, sp0v, sp1v, extras=None):
                # extras: queue of thunks (interleaved stage-1 mh1 batches),
                # emitted mid-chain at gi points where the OTHER psum tag
                # pair (previous chain's, already evacuated) is free
                extras = list(extras or [])
                for p in range(2):
                    tsolo, kbs, soff = _SOLO[p]
                    spv = sp0v if p == 0 else sp1v
                    spj = 0 if p == 0 else HALO + soff   # j base in SP tile
                    # half-sweep of 4 m-chunks = 2 psum tiles (2 chunks per
                    # tile at bank-aligned halves) -> runs of 4 same-lhsT
                    # matmuls; tags (0,1)/(2,3) alternate across half-sweeps
                    for hs in range(2):
                        tags = (2 * hs, 2 * hs + 1)
                        otags = (2 * (1 - hs), 2 * (1 - hs) + 1)
                        p2s = [psp.tile([128, 1024], f32, tag=f"ps{t}",
                                        name=f"ps{t}") for t in tags]
                        oacc = opool.tile([CO, 4 * 8 * W], mmdt,
                                          tag=f"oacc{(mh * 2 + hs) % 3}")
                        # [o, m-local, jn-pair, parity]
                        oa4 = oacc[:].rearrange("p (m j q) -> p m j q",
                                                m=32, q=2)
                        # chain: s4 solo (K=64 from u), 10 tap-pairs, then
                        # the two s-paired solos LAST (max SP-fill slack)
                        chain = [("s4",)] + \
                            [("pair", s, g) for s in range(KH)
                             for g in range(2)] + \
                            [("sp", 0), ("sp", 1)]
                        for gi, grp in enumerate(chain):
                            if grp[0] == "pair":
                                s, g = grp[1], grp[2]
                                off, kb, kk = _PAIRS[p][g]
                                bi = s * 2 + g
                            elif grp[0] == "sp":
                                bi = 10 + grp[1]
                                kb, kk = 0, 128
                            else:
                                off, kb, kk = soff, kbs, 64
                                bi = 12
                            gb = p * NG + bi
                            lhsT = wt[kb:kb + kk, gb * CO:(gb + 1) * CO]
                            for k in range(4):
                                mc = hs * 4 + k
                                if grp[0] == "sp":
                                    rhs = spv[:, mh, grp[1],
                                              spj:spj + JP,
                                              mc * 8:(mc + 1) * 8]
                                elif grp[0] == "s4":
                                    rhs = u4[kb:kb + kk, 4,
                                             HALO + soff:HALO + soff + JP,
                                             mc * 8:(mc + 1) * 8]
                                else:
                                    rhs = u4[kb:kb + kk, s,
                                             HALO + off:HALO + off + JP,
                                             mc * 8:(mc + 1) * 8]
                                mm(p2s[k // 2][:, (k % 2) * 512:
                                               (k % 2) * 512 + 512],
                                   lhsT, rhs, start=(gi == 0),
                                   stop=(gi == NG - 1), reload=(k == 0))
                            # interleave two stage-1 mh1 batches per
                            # point on the free (previous chain's, already-
                            # evacuated) psum tag pair; the final chain uses
                            # early points so its last batch's evac lands
                            # well before stage2(mh1) starts
                            pts = (5, 6, 7, 8) if (p == 1 and hs == 1) \
                                else (5, 7, 9, 11)
                            if extras and gi in pts:
                                extras.pop(0)(otags[0])
                                if extras:
                                    extras.pop(0)(otags[1])
                        evacs, trigs = [], []
                        fin = (mh == 1 and p == 1 and hs == 1)
                        for k in range(4):
                            mc = hs * 4 + k
                            psl = (p2s[k // 2][:, (k % 2) * 512:
                                               (k % 2) * 512 + 512]
                                   .rearrange("p (j m) -> p j m", j=JP))
                            # very last psum chunk: halve the evac across
                            # both engines + both DMA queues to shorten the
                            # kernel tail
                            halves = ((0, 4), (4, 8)) if (fin and k == 3) \
                                else ((0, 8),)
                            for hi, (m0, m1) in enumerate(halves):
                                lm = k * 8 + m0
                                tgt = oa4[:, lm:lm + (m1 - m0), :, p]
                                src = psl[:, :, m0:m1].transpose([0, 2, 1])
                                if (k + hi) % 2 == 0:
                                    evacs.append(lambda t=tgt, s=src:
                                                 nc.scalar.activation(
                                                     t, s,
                                                     _mb.ActivationFunctionType
                                                     .Identity, bias=bt[:]))
                                else:
                                    evacs.append(lambda t=tgt, s=src:
                                                 nc.vector.tensor_scalar_add(
                                                     t, s, bt[:]))
                                if p == 1:   # both parities done -> stream;
                                    # triggers split across HWDGE queues,
                                    # emitted after ALL evacs so the scalar
                                    # engine never delays an evac for one
                                    deng = nc.sync if (k + hi) % 2 == 0 \
                                        else nc.scalar
                                    trigs.append(
                                        lambda d=deng, mc=mc, lm=lm,
                                        m0=m0, m1=m1:
                                        d.dma_start(
                                            o_d[:, mh * MH + mc * 8 + m0:
                                                mh * MH + mc * 8 + m1, :],
                                            oacc[:, lm * W:
                                                 (lm + m1 - m0) * W]))
                        for e in evacs:
                            e()
                        for t in trigs:
                            t()
                assert not extras

            u = upool.tile([128, 2 * KH * JX * MH], mmdt)
            u5 = u[:].rearrange("p (h s j m) -> p h s j m", h=2, s=KH, j=JX)

            # SP tiles: s-paired (z = s parity across partition halves)
            # copies of u's solo-tap data, DMA-filled per mh half once that
            # half's stage-1 writes land.  SP0: d=0 (p=0 solo, t=0, no
            # halo); SP1: d=1 (p=1 solo, t=4, keeps front halo).
            sp0 = upool.tile([128, 2 * 2 * JP * MH], mmdt)
            sp1 = upool.tile([128, 2 * 2 * JX * MH], mmdt)
            sp0v = sp0[:].rearrange("p (h q j m) -> p h q j m", h=2, q=2, j=JP)
            sp1v = sp1[:].rearrange("p (h q j m) -> p h q j m", h=2, q=2, j=JX)

            if MH_SPLIT:
                # phase 1: stage-1 for mh0 only (evac-throughput-bound
                # window is half as long); mh1's stage-1 batches run INSIDE
                # stage2(mh0)'s chains where the evac engines have slack
                for bn, (ja, jb) in enumerate(S1BATCHES):
                    s1_batch(0, ja, jb, bn % 4, bn % 2)
                sp_fills(0, sp0v, sp1v)
                mh1_thunks = [
                    (lambda t, ja=ja, jb=jb, e=bn % 2:
                     s1_batch(1, ja, jb, t, e))
                    for bn, (ja, jb) in enumerate(S1BATCHES)]
                stage2(u5[:, 0], 0, sp0v, sp1v, extras=mh1_thunks)
                sp_fills(1, sp0v, sp1v)
                stage2(u5[:, 1], 1, sp0v, sp1v)
            else:
                for n, jp in enumerate(JPORDER):
                    s1_jp(jp, n % 4, n % 2)
                sp_fills(0, sp0v, sp1v)
                sp_fills(1, sp0v, sp1v)
                stage2(u5[:, 0], 0, sp0v, sp1v)
                stage2(u5[:, 1], 1, sp0v, sp1v)
    _strip_redundant_ldweights(nc)
    nc.compile()
    return nc


def _strip_redundant_ldweights(nc):
    """Drop InstLdweights whose weights AP equals the previous load on the
    tensor queue: the PE array still holds those weights (nothing between
    two same-AP loads rewrites that SBUF region in this kernel), so the
    reload is pure overhead (~100ns each, serialized with the matmuls)."""

    def sig(ap):
        return str(ap)

    removed = kept = 0
    for fn in nc.m.functions:
        for bb in fn.blocks:
            insts = bb.instructions
            last = None
            for inst in list(insts):
                nm = type(inst).__name__
                if nm == "InstLdweights":
                    s = sig(inst.ins[0])
                    si = inst.sync_info
                    clean = si is None or (not si.on_wait and not si.on_update)
                    if s == last and clean:
                        insts.remove(inst)
                        removed += 1
                    else:
                        last = s
                        kept += 1
                elif nm == "InstMatmult":
                    pass          # matmuls leave the stationary weights alone
    return removed, kept


def _get_prog():
    global _PROG
    if _PROG is None:
        _PROG = _build_program()
    return _PROG


def kernel(x, weight, bias):
    from concourse.bass_utils import run_bass_kernel_spmd

    global _CONSTS
    if _CONSTS is None:
        _CONSTS = _build_consts()
    GT = _CONSTS

    x = np.ascontiguousarray(np.asarray(x, dtype=np.float32))
    weight = np.ascontiguousarray(np.asarray(weight, dtype=np.float32))
    bias = np.ascontiguousarray(np.asarray(bias, dtype=np.float32))

    wst = _build_wstack(weight)
    b2 = np.ascontiguousarray(bias.reshape(CO, 1))

    in_maps = []
    for b in range(B):
        # x2[h, (w, i)] with jp col-blocks PERMUTED into JPORDER order so
        # DMA chunks stream in exact device consumption order
        x2 = (x[b].transpose(1, 2, 0).reshape(H, JP, 2 * CI)
              [:, list(JPORDER), :].reshape(H, W * CI)).astype(_np_dt())
        m = {}
        for c, (off, sz) in enumerate(XCHUNKS):
            m[f"x{c}"] = np.ascontiguousarray(x2[:, off:off + sz])
        m.update({"g0a": np.ascontiguousarray(GT[:, 0:128]),
                  "g0b": np.ascontiguousarray(GT[:, 128:KH * MH]),
                  "g1": np.ascontiguousarray(GT[:, KH * MH:]),
                  "wt": wst, "bias": b2})
        in_maps.append(m)

    res = run_bass_kernel_spmd(_get_prog(), in_maps, core_ids=list(range(B)),
                               **_RUN_OPTS)
    global _LAST_RESULT
    _LAST_RESULT = res
    out = np.stack([res.results[b]["out"] for b in range(B)], axis=0)
    return np.ascontiguousarray(out.astype(np.float32))

